# revision 10
# baseline (speedup 1.0000x reference)
"""Trainium2 Bass kernel for nn_FactorizedEnsembleModel.

Reference computation (D=18, E=10, IN=23, H=128, B=4096):
    m  = transpose(masks, (1,0,2))                      # (D,E,IN)
    xm = x * m  (broadcast over batch)                  # (D,E,B,IN)
    h1 = silu(xm @ W1 + b1)                             # (D,E,B,H)
    h2 = silu(h1 @ W2 + b2)                             # (D,E,B,H)
    out = h2 @ W3 + b3                                  # (D,E,B,2)
    mean, logvar = out[...,0:1], out[...,1:2]
    logvar = MAX - softplus(MAX - logvar)
    logvar = MIN + softplus(logvar - MIN)
    returns (mean, logvar), each (D,E,B,1)

Sharding: data-parallel over batch, B=4096 -> 512 per core across 8 cores.
Every core runs all 180 (d,e) expert MLPs on its batch slice.

Device mapping per (d,e) pair (fp32r matmuls, N=512):
    mm1: lhsT = [mask*W1; b1] (24,128), rhs = [x^T; ones] (24,512) -> psum(128,512)
    silu on ACT -> h1 sbuf
    mm2: lhsT = W2 (128,128), rhs = h1 -> psum(128,512)
    silu(. + b2) on ACT (per-partition bias) -> h2 sbuf
    mm3 transposed: for each 128-batch chunk c, lhsT = h2[:,c] (128,128),
        rhs = W3 (128,2) -> psum[128, 2c:2c+2]; psum tile (128,8) has batch
        on partitions and (chunk, mean/lv) on the free axis.
    one DVE copy (128,8) -> staging tile (128, 180*8)
Tail phase, batched over all pairs (batch on partitions, pairs on free):
    s = staging + broadcast(b3); lv-columns get the double-softplus clamp
    with constant biases; single contiguous DMA of (128, 1440) out.
Host reassembles (pair, chunk, batch-row) -> (D,E,B,1).
"""

import sys

import numpy as np

if "/opt/trn_rl_repo" not in sys.path:
    sys.path.insert(0, "/opt/trn_rl_repo")

D, E, IN, H, B = 18, 10, 23, 128, 4096
P = D * E  # 180 expert pairs
NCORES = 8
BL = B // NCORES  # 512 batch per core
NCHUNK = BL // 128  # 4 batch chunks of 128
FREE = P * NCHUNK * 2  # 1440 staging columns
MIN_LOGVAR = -10.0
MAX_LOGVAR = 5.0

PROFILE = False  # test.py flips this to capture an NTFF trace
LAST_RESULT = None  # BassKernelResults from the most recent run

_NC_CACHE = {}


def build_bass():
    import concourse.bass as bass
    import concourse.mybir as mybir
    import concourse.tile as tile
    from concourse import bacc

    FP = mybir.dt.float32
    FR = mybir.dt.float32r
    AF = mybir.ActivationFunctionType
    ALU = mybir.AluOpType

    nc = bacc.Bacc(None)

    xTa_d = nc.dram_tensor("xTa", [IN + 1, BL], FR, kind="ExternalInput")
    w1_d = nc.dram_tensor("w1", [P, IN + 1, H], FR, kind="ExternalInput")
    w2_d = nc.dram_tensor("w2", [P, H, H], FR, kind="ExternalInput")
    w3_d = nc.dram_tensor("w3", [P, H, 2], FR, kind="ExternalInput")
    b2T_d = nc.dram_tensor("b2T", [H, P], FP, kind="ExternalInput")
    b3row_d = nc.dram_tensor("b3row", [1, FREE], FP, kind="ExternalInput")
    out_o = nc.dram_tensor("out", [128, FREE], FP, kind="ExternalOutput")

    with tile.TileContext(nc) as tc:
        with (
            tc.tile_pool(name="consts", bufs=1) as consts,
            tc.tile_pool(name="wpool", bufs=4) as wpool,
            tc.tile_pool(name="hpool", bufs=3) as hpool,
            tc.tile_pool(name="pspool", bufs=2, space="PSUM") as pspool,
            tc.tile_pool(name="ps3pool", bufs=3, space="PSUM") as ps3pool,
            tc.tile_pool(name="tailpool", bufs=1) as tailpool,
        ):
            xTa = consts.tile([IN + 1, BL], FR)
            nc.sync.dma_start(xTa, xTa_d[:, :])
            b2T = consts.tile([H, P], FP)
            nc.sync.dma_start(b2T, b2T_d[:, :])
            b3bc = consts.tile([128, FREE], FP)
            nc.sync.dma_start(
                b3bc,
                bass.AP(
                    tensor=b3row_d[:].tensor,
                    offset=b3row_d[:].offset,
                    ap=[[0, 128], [1, FREE]],
                ),
            )
            stg = consts.tile([128, FREE], FP)

            for p in range(P):
                w1 = wpool.tile([IN + 1, H], FR, tag="w1")
                nc.sync.dma_start(w1, w1_d[p])
                w2 = wpool.tile([H, H], FR, tag="w2")
                nc.sync.dma_start(w2, w2_d[p])
                w3 = wpool.tile([H, 2], FR, tag="w3")
                nc.sync.dma_start(w3, w3_d[p])

                ps1 = pspool.tile([H, BL], FP, tag="ps1")
                nc.tensor.matmul(
                    ps1, lhsT=w1, rhs=xTa, start=True, stop=True
                )
                h1 = hpool.tile([H, BL], FR, tag="h1")
                nc.scalar.activation(h1, ps1, AF.Silu)

                ps2 = pspool.tile([H, BL], FP, tag="ps2")
                nc.tensor.matmul(
                    ps2, lhsT=w2, rhs=h1, start=True, stop=True
                )
                h2 = hpool.tile([H, BL], FR, tag="h2")
                nc.scalar.activation(h2, ps2, AF.Silu, bias=b2T[:, p : p + 1], scale=1.0)

                # mm3 transposed: batch chunks on partitions, (chunk,o) on free
                ps3 = ps3pool.tile([128, 2 * NCHUNK], FP, tag="ps3")
                for c in range(NCHUNK):
                    nc.tensor.matmul(
                        ps3[:, 2 * c : 2 * c + 2],
                        lhsT=h2[:, 128 * c : 128 * (c + 1)],
                        rhs=w3,
                        start=True,
                        stop=True,
                    )
                nc.vector.tensor_copy(
                    stg[:, p * 2 * NCHUNK : (p + 1) * 2 * NCHUNK], ps3
                )

            # Tail: add b3 everywhere, then clamp the logvar columns in place.
            # softplus(z) computed stably as max(z,0) + ln(1 + exp(-|z|))
            # (no Softplus activation table in this toolchain; Exp and Ln
            # share one table set).
            HF = FREE // 2

            def emit_softplus(z, tag):
                a = tailpool.tile([128, HF], FP, tag=f"{tag}_a")
                # |z| = max(-z, z)
                nc.vector.scalar_tensor_tensor(a, z, -1.0, z, ALU.mult, ALU.max)
                e = tailpool.tile([128, HF], FP, tag=f"{tag}_e")
                nc.scalar.activation(e, a, AF.Exp, scale=-1.0)  # exp(-|z|)
                l = tailpool.tile([128, HF], FP, tag=f"{tag}_l")
                nc.scalar.activation(l, e, AF.Ln, bias=1.0, scale=1.0)  # ln(1+u)
                mx = tailpool.tile([128, HF], FP, tag=f"{tag}_m")
                nc.vector.tensor_scalar_max(mx, z, 0.0)
                o = tailpool.tile([128, HF], FP, tag=f"{tag}_o")
                nc.vector.tensor_add(o, l, mx)
                return o

            s = tailpool.tile([128, FREE], FP, tag="s")
            nc.vector.tensor_add(s, stg, b3bc)
            lv_v = s.rearrange("b (f o) -> b f o", o=2)[:, :, 1]  # (128, 720)
            # z1 = MAX - lv
            z1 = tailpool.tile([128, HF], FP, tag="z1")
            nc.vector.tensor_scalar(z1, lv_v, -1.0, MAX_LOGVAR, ALU.mult, ALU.add)
            t1 = emit_softplus(z1, "sp1")
            # z2 = (MAX - t1) - MIN
            z2 = tailpool.tile([128, HF], FP, tag="z2")
            nc.vector.tensor_scalar(
                z2, t1, -1.0, MAX_LOGVAR - MIN_LOGVAR, ALU.mult, ALU.add
            )
            t3 = emit_softplus(z2, "sp2")
            # write back: lv = MIN + t3
            nc.vector.tensor_scalar_add(lv_v, t3, MIN_LOGVAR)
            nc.sync.dma_start(out_o[:, :], s)

    nc.compile()
    return nc


def _get_nc():
    if "nc" not in _NC_CACHE:
        _NC_CACHE["nc"] = build_bass()
    return _NC_CACHE["nc"]


def host_prep(x, masks, W1, b1, W2, b2, W3, b3):
    """Numpy-side input massaging shared by kernel() and the simulator test."""
    f32 = np.float32
    x = np.asarray(x, f32)
    masks = np.asarray(masks, f32)
    W1 = np.asarray(W1, f32)
    b1 = np.asarray(b1, f32)
    W2 = np.asarray(W2, f32)
    b2 = np.asarray(b2, f32)
    W3 = np.asarray(W3, f32)
    b3 = np.asarray(b3, f32)

    m = masks.transpose(1, 0, 2)  # (D,E,IN)
    W1m = m[:, :, :, None] * W1  # (D,E,IN,H): (x*m)@W1 == x@(m*W1)
    W1a = np.concatenate([W1m, b1[:, :, None, :]], axis=2)  # (D,E,IN+1,H)
    w1 = np.ascontiguousarray(W1a.reshape(P, IN + 1, H))
    w2 = np.ascontiguousarray(W2.reshape(P, H, H))
    w3 = np.ascontiguousarray(W3.reshape(P, H, 2))
    b2T = np.ascontiguousarray(b2.reshape(P, H).T)  # (H,P)
    b3r = b3.reshape(P, 2)
    # broadcast row: index p*8 + c*2 + o -> b3r[p, o]
    b3row = np.ascontiguousarray(
        np.tile(b3r[:, None, :], (1, NCHUNK, 1)).reshape(1, FREE)
    )

    xT = np.ascontiguousarray(x.T)  # (IN,B)
    per_core = []
    for c in range(NCORES):
        sl = xT[:, c * BL : (c + 1) * BL]
        xTa = np.concatenate([sl, np.ones((1, BL), f32)], axis=0)  # (IN+1,BL)
        per_core.append(np.ascontiguousarray(xTa))

    common = {"w1": w1, "w2": w2, "w3": w3, "b2T": b2T, "b3row": b3row}
    return common, per_core


def assemble(core_outs):
    """core_outs: list of (128, FREE) arrays -> (mean, logvar), (D,E,nb,1)."""
    slices = []
    for arr in core_outs:
        a = arr.reshape(128, P, NCHUNK, 2)  # (b, pair, chunk, o)
        a = a.transpose(1, 2, 0, 3).reshape(P, BL, 2)  # (pair, chunk*128+b, o)
        slices.append(a)
    full = np.concatenate(slices, axis=1)  # (P, nb, 2)
    nb = full.shape[1]
    mean = full[:, :, 0].reshape(D, E, nb, 1).astype(np.float32)
    lv = full[:, :, 1].reshape(D, E, nb, 1).astype(np.float32)
    return mean, lv


def kernel(x, masks, W1, b1, W2, b2, W3, b3):
    global LAST_RESULT
    from concourse.bass_utils import run_bass_kernel_spmd

    common, per_core = host_prep(x, masks, W1, b1, W2, b2, W3, b3)
    nc = _get_nc()

    in_maps = [dict(common, xTa=per_core[c]) for c in range(NCORES)]
    res = run_bass_kernel_spmd(
        nc,
        in_maps,
        core_ids=list(range(NCORES)),
        trace=PROFILE,
    )
    LAST_RESULT = res

    return assemble([r["out"] for r in res.results])


# revision 13
# speedup vs baseline: 1.2587x; 1.2587x over previous
"""Trainium2 Bass kernel for nn_FactorizedEnsembleModel.

Reference computation (D=18, E=10, IN=23, H=128, B=4096):
    m  = transpose(masks, (1,0,2))                      # (D,E,IN)
    xm = x * m  (broadcast over batch)                  # (D,E,B,IN)
    h1 = silu(xm @ W1 + b1)                             # (D,E,B,H)
    h2 = silu(h1 @ W2 + b2)                             # (D,E,B,H)
    out = h2 @ W3 + b3                                  # (D,E,B,2)
    mean, logvar = out[...,0:1], out[...,1:2]
    logvar = MAX - softplus(MAX - logvar)
    logvar = MIN + softplus(logvar - MIN)
    returns (mean, logvar), each (D,E,B,1)

Sharding: data-parallel over batch, B=4096 -> 512 per core across 8 cores.
Every core runs all 180 (d,e) expert MLPs on its batch slice.

Device mapping per (d,e) pair (fp32r matmuls, N=512):
    mm1: lhsT = [mask*W1; b1] (24,128), rhs = [x^T; ones] (24,512) -> psum(128,512)
    silu on ACT -> h1 sbuf
    mm2: lhsT = W2 (128,128), rhs = h1 -> psum(128,512)
    silu(. + b2) on ACT (per-partition bias) -> h2 sbuf
    mm3: lhsT = W3 (128,2), rhs = h2 -> psum(2,512)   [LDW is 2 cols: cheap]
    DVE copy psum(2,512) -> per-group tmp; grouped DMA scatters rows into
    staging tiles stg_m/stg_l (128, 1024) with partition = pair%128,
    column block = pair//128.
Tail phase per column-block (pairs on partitions):
    mean += b3_mean (per-partition bias);
    logvar: z1 = (MAX - b3_lv) - lv ; double softplus clamp with
    softplus(z) = max(z,0) + ln(1 + exp(-|z|))  (Exp + Ln share one
    activation table set; no native Softplus table in this toolchain).
Host reassembles (pair, batch) -> (D,E,B,1).
"""

import sys

import numpy as np

if "/opt/trn_rl_repo" not in sys.path:
    sys.path.insert(0, "/opt/trn_rl_repo")

D, E, IN, H, B = 18, 10, 23, 128, 4096
P = D * E  # 180 expert pairs
NCORES = 8
BL = B // NCORES  # 512 batch per core
NBLK = (P + 127) // 128  # 2 staging column blocks
G = 4  # pairs per staging group (must divide 128)
W2CH = 12  # pairs per W2 DMA chunk
MIN_LOGVAR = -10.0
MAX_LOGVAR = 5.0

PROFILE = False  # test.py flips this to capture an NTFF trace
LAST_RESULT = None  # BassKernelResults from the most recent run

_NC_CACHE = {}


def build_bass():
    import concourse.mybir as mybir
    import concourse.tile as tile
    from concourse import bacc

    FP = mybir.dt.float32
    FR = mybir.dt.float32r
    AF = mybir.ActivationFunctionType
    ALU = mybir.AluOpType

    nc = bacc.Bacc(None)

    xTa_d = nc.dram_tensor("xTa", [IN + 1, BL], FR, kind="ExternalInput")
    w1_d = nc.dram_tensor("w1", [IN + 1, P * H], FR, kind="ExternalInput")
    w2_d = nc.dram_tensor("w2", [H, P * H], FR, kind="ExternalInput")
    w3_d = nc.dram_tensor("w3", [H, 2 * P], FR, kind="ExternalInput")
    b2T_d = nc.dram_tensor("b2T", [H, P], FP, kind="ExternalInput")
    b3m_d = nc.dram_tensor("b3m", [P, 1], FP, kind="ExternalInput")
    sb1_d = nc.dram_tensor("sb1", [P, 1], FP, kind="ExternalInput")  # MAX - b3_lv
    mean_o = nc.dram_tensor("mean", [128, NBLK * BL], FP, kind="ExternalOutput")
    lv_o = nc.dram_tensor("lv", [128, NBLK * BL], FP, kind="ExternalOutput")

    with tile.TileContext(nc) as tc:
        with (
            tc.tile_pool(name="consts", bufs=1) as consts,
            tc.tile_pool(name="w2pool", bufs=3) as w2pool,
            tc.tile_pool(name="hpool", bufs=3) as hpool,
            tc.tile_pool(name="tmppool", bufs=2) as tmppool,
            tc.tile_pool(name="pspool", bufs=2, space="PSUM") as pspool,
            tc.tile_pool(name="ps3pool", bufs=2, space="PSUM") as ps3pool,
            tc.tile_pool(name="tailpool", bufs=1) as tailpool,
        ):
            xTa = consts.tile([IN + 1, BL], FR)
            nc.sync.dma_start(xTa, xTa_d[:, :])
            b2T = consts.tile([H, P], FP)
            nc.sync.dma_start(b2T, b2T_d[:, :])
            w3all = consts.tile([H, 2 * P], FR)
            nc.sync.dma_start(w3all, w3_d[:, :])
            w1all = consts.tile([IN + 1, P * H], FR)
            for c in range(4):
                cs = c * (P // 4) * H
                ce = (c + 1) * (P // 4) * H
                nc.sync.dma_start(w1all[:, cs:ce], w1_d[:, cs:ce])
            stg_m = consts.tile([128, NBLK * BL], FP)
            stg_l = consts.tile([128, NBLK * BL], FP)

            tmp = None
            for p in range(P):
                ci = p % W2CH
                if ci == 0:
                    npair = min(W2CH, P - p)
                    w2c = w2pool.tile([H, W2CH * H], FR, tag="w2c")
                    nc.sync.dma_start(
                        w2c[:, : npair * H], w2_d[:, p * H : (p + npair) * H]
                    )

                ps1 = pspool.tile([H, BL], FP, tag="ps1")
                nc.tensor.matmul(
                    ps1,
                    lhsT=w1all[:, p * H : (p + 1) * H],
                    rhs=xTa,
                    start=True,
                    stop=True,
                )
                h1 = hpool.tile([H, BL], FR, tag="h1")
                nc.scalar.activation(h1, ps1, AF.Silu)

                ps2 = pspool.tile([H, BL], FP, tag="ps2")
                nc.tensor.matmul(
                    ps2,
                    lhsT=w2c[:, ci * H : (ci + 1) * H],
                    rhs=h1,
                    start=True,
                    stop=True,
                )
                h2 = hpool.tile([H, BL], FR, tag="h2")
                nc.scalar.activation(h2, ps2, AF.Silu, bias=b2T[:, p : p + 1], scale=1.0)

                ps3 = ps3pool.tile([2, BL], FP, tag="ps3")
                nc.tensor.matmul(
                    ps3, lhsT=w3all[:, 2 * p : 2 * p + 2], rhs=h2, start=True, stop=True
                )

                gi = p % G
                if gi == 0:
                    tmp = tmppool.tile([2, G * BL], FP, tag="tmp")
                nc.vector.tensor_copy(tmp[:, gi * BL : (gi + 1) * BL], ps3)
                if gi == G - 1:
                    g0 = p - G + 1  # first pair of the group
                    r0 = g0 % 128
                    cs = (g0 // 128) * BL
                    src_m = tmp[0:1, :].rearrange("a (g b) -> a g b", b=BL)
                    src_l = tmp[1:2, :].rearrange("a (g b) -> a g b", b=BL)
                    nc.sync.dma_start(stg_m[r0 : r0 + G, cs : cs + BL], src_m)
                    nc.sync.dma_start(stg_l[r0 : r0 + G, cs : cs + BL], src_l)

            # Tail: per column-block, bias-add mean and clamp logvar.
            b3m_t = consts.tile([128, NBLK], FP)
            sb1_t = consts.tile([128, NBLK], FP)
            for blk in range(NBLK):
                n = min(128, P - blk * 128)
                nc.sync.dma_start(
                    b3m_t[:n, blk : blk + 1], b3m_d[blk * 128 : blk * 128 + n]
                )
                nc.sync.dma_start(
                    sb1_t[:n, blk : blk + 1], sb1_d[blk * 128 : blk * 128 + n]
                )

            def emit_softplus(z, n, tag):
                # softplus(z) = max(z,0) + ln(1 + exp(-|z|))
                a = tailpool.tile([128, BL], FP, tag=f"{tag}_a")
                nc.vector.scalar_tensor_tensor(
                    a[:n], z, -1.0, z, ALU.mult, ALU.max
                )  # |z|
                e = tailpool.tile([128, BL], FP, tag=f"{tag}_e")
                nc.scalar.activation(e[:n], a[:n], AF.Exp, scale=-1.0)
                l = tailpool.tile([128, BL], FP, tag=f"{tag}_l")
                nc.scalar.activation(l[:n], e[:n], AF.Ln, bias=1.0, scale=1.0)
                mx = tailpool.tile([128, BL], FP, tag=f"{tag}_m")
                nc.vector.tensor_scalar_max(mx[:n], z, 0.0)
                o = tailpool.tile([128, BL], FP, tag=f"{tag}_o")
                nc.vector.tensor_add(o[:n], l[:n], mx[:n])
                return o[:n]

            for blk in range(NBLK):
                n = min(128, P - blk * 128)
                cs = blk * BL
                mout = tailpool.tile([128, BL], FP, tag="mout")
                nc.vector.tensor_scalar_add(
                    mout[:n], stg_m[:n, cs : cs + BL], b3m_t[:n, blk : blk + 1]
                )
                nc.sync.dma_start(mean_o[:n, cs : cs + BL], mout[:n])

                # z1 = (MAX - b3_lv) - lv
                z1 = tailpool.tile([128, BL], FP, tag="z1")
                nc.vector.tensor_scalar(
                    z1[:n],
                    stg_l[:n, cs : cs + BL],
                    -1.0,
                    sb1_t[:n, blk : blk + 1],
                    ALU.mult,
                    ALU.add,
                )
                t1 = emit_softplus(z1[:n], n, "sp1")
                # z2 = (MAX - t1) - MIN
                z2 = tailpool.tile([128, BL], FP, tag="z2")
                nc.vector.tensor_scalar(
                    z2[:n], t1, -1.0, MAX_LOGVAR - MIN_LOGVAR, ALU.mult, ALU.add
                )
                t3 = emit_softplus(z2[:n], n, "sp2")
                lvout = tailpool.tile([128, BL], FP, tag="lvout")
                nc.vector.tensor_scalar_add(lvout[:n], t3, MIN_LOGVAR)
                nc.sync.dma_start(lv_o[:n, cs : cs + BL], lvout[:n])

    nc.compile()
    return nc


def _get_nc():
    if "nc" not in _NC_CACHE:
        _NC_CACHE["nc"] = build_bass()
    return _NC_CACHE["nc"]


def host_prep(x, masks, W1, b1, W2, b2, W3, b3):
    """Numpy-side input massaging shared by kernel() and the simulator test."""
    f32 = np.float32
    x = np.asarray(x, f32)
    masks = np.asarray(masks, f32)
    W1 = np.asarray(W1, f32)
    b1 = np.asarray(b1, f32)
    W2 = np.asarray(W2, f32)
    b2 = np.asarray(b2, f32)
    W3 = np.asarray(W3, f32)
    b3 = np.asarray(b3, f32)

    m = masks.transpose(1, 0, 2)  # (D,E,IN)
    W1m = m[:, :, :, None] * W1  # (D,E,IN,H): (x*m)@W1 == x@(m*W1)
    W1a = np.concatenate([W1m, b1[:, :, None, :]], axis=2)  # (D,E,IN+1,H)
    w1 = np.ascontiguousarray(
        W1a.reshape(P, IN + 1, H).transpose(1, 0, 2).reshape(IN + 1, P * H)
    )
    w2 = np.ascontiguousarray(
        W2.reshape(P, H, H).transpose(1, 0, 2).reshape(H, P * H)
    )
    w3 = np.ascontiguousarray(
        W3.reshape(P, H, 2).transpose(1, 0, 2).reshape(H, 2 * P)
    )
    b2T = np.ascontiguousarray(b2.reshape(P, H).T)  # (H,P)
    b3r = b3.reshape(P, 2)
    b3m = np.ascontiguousarray(b3r[:, 0:1])  # (P,1)
    sb1 = np.ascontiguousarray(MAX_LOGVAR - b3r[:, 1:2])  # (P,1)

    xT = np.ascontiguousarray(x.T)  # (IN,B)
    per_core = []
    for c in range(NCORES):
        sl = xT[:, c * BL : (c + 1) * BL]
        xTa = np.concatenate([sl, np.ones((1, BL), f32)], axis=0)  # (IN+1,BL)
        per_core.append(np.ascontiguousarray(xTa))

    common = {"w1": w1, "w2": w2, "w3": w3, "b2T": b2T, "b3m": b3m, "sb1": sb1}
    return common, per_core


def assemble(core_means, core_lvs):
    """(128, NBLK*BL) staging dumps per core -> (mean, logvar), (D,E,nb,1)."""

    def unstage(arr):
        # pair p lives at [p % 128, (p // 128)*BL : ...]
        blocks = [arr[:, b * BL : (b + 1) * BL] for b in range(NBLK)]
        return np.concatenate(blocks, axis=0)[:P]  # (P, BL)

    mean = np.concatenate([unstage(a) for a in core_means], axis=1)  # (P, nb)
    lv = np.concatenate([unstage(a) for a in core_lvs], axis=1)
    nb = mean.shape[1]
    mean = mean.reshape(D, E, nb, 1).astype(np.float32)
    lv = lv.reshape(D, E, nb, 1).astype(np.float32)
    return mean, lv


def kernel(x, masks, W1, b1, W2, b2, W3, b3):
    global LAST_RESULT
    from concourse.bass_utils import run_bass_kernel_spmd

    common, per_core = host_prep(x, masks, W1, b1, W2, b2, W3, b3)
    nc = _get_nc()

    in_maps = [dict(common, xTa=per_core[c]) for c in range(NCORES)]
    res = run_bass_kernel_spmd(
        nc,
        in_maps,
        core_ids=list(range(NCORES)),
        trace=PROFILE,
    )
    LAST_RESULT = res

    return assemble(
        [r["mean"] for r in res.results], [r["lv"] for r in res.results]
    )


# revision 15
# speedup vs baseline: 1.8370x; 1.4594x over previous
"""Trainium2 Bass kernel for nn_FactorizedEnsembleModel.

Reference computation (D=18, E=10, IN=23, H=128, B=4096):
    m  = transpose(masks, (1,0,2))                      # (D,E,IN)
    xm = x * m  (broadcast over batch)                  # (D,E,B,IN)
    h1 = silu(xm @ W1 + b1)                             # (D,E,B,H)
    h2 = silu(h1 @ W2 + b2)                             # (D,E,B,H)
    out = h2 @ W3 + b3                                  # (D,E,B,2)
    mean, logvar = out[...,0:1], out[...,1:2]
    logvar = MAX - softplus(MAX - logvar)
    logvar = MIN + softplus(logvar - MIN)
    returns (mean, logvar), each (D,E,B,1)

Sharding: data-parallel over batch, B=4096 -> 512 per core across 8 cores.
Every core runs all 180 (d,e) expert MLPs on its batch slice.

Device mapping per (d,e) pair (fp32r matmuls, N=512):
    mm1: lhsT = [mask*W1; b1] (24,128), rhs = [x^T; ones] (24,512) -> psum(128,512)
    silu on ACT -> h1 sbuf
    mm2: lhsT = W2 (128,128), rhs = h1 -> psum(128,512)
    silu(. + b2) on ACT (per-partition bias) -> h2 sbuf
    mm3: lhsT = W3 (128,2), rhs = h2 -> psum(2,512)   [LDW is 2 cols: cheap]
    DVE copy psum(2,512) -> per-group tmp; grouped DMA scatters rows into
    staging tiles stg_m/stg_l (128, 1024) with partition = pair%128,
    column block = pair//128.
Tail phase per column-block (pairs on partitions):
    mean += b3_mean (per-partition bias);
    logvar: z1 = (MAX - b3_lv) - lv ; double softplus clamp with
    softplus(z) = max(z,0) + ln(1 + exp(-|z|))  (Exp + Ln share one
    activation table set; no native Softplus table in this toolchain).
Host reassembles (pair, batch) -> (D,E,B,1).
"""

import sys

import numpy as np

if "/opt/trn_rl_repo" not in sys.path:
    sys.path.insert(0, "/opt/trn_rl_repo")

D, E, IN, H, B = 18, 10, 23, 128, 4096
P = D * E  # 180 expert pairs
NCORES = 8
BL = B // NCORES  # 512 batch per core
NBLK = (P + 127) // 128  # 2 staging column blocks
G = 4  # pairs per staging group (must divide 128)
W2CH = 12  # pairs per W2 DMA chunk
MIN_LOGVAR = -10.0
MAX_LOGVAR = 5.0

PROFILE = False  # test.py flips this to capture an NTFF trace
LAST_RESULT = None  # BassKernelResults from the most recent run

_NC_CACHE = {}


def build_bass():
    import concourse.mybir as mybir
    import concourse.tile as tile
    from concourse import bacc

    FP = mybir.dt.float32
    FR = mybir.dt.float32r
    AF = mybir.ActivationFunctionType
    ALU = mybir.AluOpType

    nc = bacc.Bacc(None)

    xTa_d = nc.dram_tensor("xTa", [IN + 1, BL], FR, kind="ExternalInput")
    w1_d = nc.dram_tensor("w1", [IN + 1, P * H], FR, kind="ExternalInput")
    w2_d = nc.dram_tensor("w2", [H, P * H], FR, kind="ExternalInput")
    w3_d = nc.dram_tensor("w3", [H, 2 * P], FR, kind="ExternalInput")
    b2T_d = nc.dram_tensor("b2T", [H, P], FP, kind="ExternalInput")
    b3m_d = nc.dram_tensor("b3m", [P, 1], FP, kind="ExternalInput")
    sb1_d = nc.dram_tensor("sb1", [P, 1], FP, kind="ExternalInput")  # MAX - b3_lv
    mean_o = nc.dram_tensor("mean", [128, NBLK * BL], FP, kind="ExternalOutput")
    lv_o = nc.dram_tensor("lv", [128, NBLK * BL], FP, kind="ExternalOutput")

    with tile.TileContext(nc) as tc:
        with (
            tc.tile_pool(name="consts", bufs=1) as consts,
            tc.tile_pool(name="w2pool", bufs=3) as w2pool,
            tc.tile_pool(name="hpool", bufs=4) as hpool,
            tc.tile_pool(name="tmppool", bufs=2) as tmppool,
            tc.tile_pool(name="pspool", bufs=3, space="PSUM") as pspool,
            tc.tile_pool(name="ps3pool", bufs=2, space="PSUM") as ps3pool,
            tc.tile_pool(name="tailpool", bufs=1) as tailpool,
        ):
            xTa = consts.tile([IN + 1, BL], FR)
            nc.sync.dma_start(xTa, xTa_d[:, :])
            b2T = consts.tile([H, P], FP)
            nc.sync.dma_start(b2T, b2T_d[:, :])
            w3all = consts.tile([H, 2 * P], FR)
            nc.sync.dma_start(w3all, w3_d[:, :])
            w1all = consts.tile([IN + 1, P * H], FR)
            for c in range(4):
                cs = c * (P // 4) * H
                ce = (c + 1) * (P // 4) * H
                nc.sync.dma_start(w1all[:, cs:ce], w1_d[:, cs:ce])
            stg_m = consts.tile([128, NBLK * BL], FP)
            stg_l = consts.tile([128, NBLK * BL], FP)

            # Software pipeline over pairs: stage offsets keep the PE
            # streaming back-to-back instead of serializing on the
            # mm1->silu1->mm2->silu2->mm3 chain within one pair.
            LAG2, LAG3 = 2, 4
            h1s = {}
            h2s = {}
            w2cs = {}
            ps3s = {}
            tmp = None
            for i in range(P + LAG3):
                p1, p2, p3 = i, i - LAG2, i - LAG3
                if p1 < P:
                    ci = p1 % W2CH
                    if ci == 0:
                        npair = min(W2CH, P - p1)
                        w2c = w2pool.tile([H, W2CH * H], FR, tag="w2c")
                        nc.sync.dma_start(
                            w2c[:, : npair * H], w2_d[:, p1 * H : (p1 + npair) * H]
                        )
                        w2cs[p1 // W2CH] = w2c
                    ps1 = pspool.tile([H, BL], FP, tag="ps1")
                    nc.tensor.matmul(
                        ps1,
                        lhsT=w1all[:, p1 * H : (p1 + 1) * H],
                        rhs=xTa,
                        start=True,
                        stop=True,
                    )
                    h1 = hpool.tile([H, BL], FR, tag="h1")
                    nc.scalar.activation(h1, ps1, AF.Silu)
                    h1s[p1] = h1
                if 0 <= p2 < P:
                    ps2 = pspool.tile([H, BL], FP, tag="ps2")
                    nc.tensor.matmul(
                        ps2,
                        lhsT=w2cs[p2 // W2CH][:, (p2 % W2CH) * H : (p2 % W2CH + 1) * H],
                        rhs=h1s.pop(p2),
                        start=True,
                        stop=True,
                    )
                    h2 = hpool.tile([H, BL], FR, tag="h2")
                    nc.scalar.activation(
                        h2, ps2, AF.Silu, bias=b2T[:, p2 : p2 + 1], scale=1.0
                    )
                    h2s[p2] = h2
                if 0 <= p3 < P:
                    ps3 = ps3pool.tile([2, BL], FP, tag="ps3")
                    nc.tensor.matmul(
                        ps3,
                        lhsT=w3all[:, 2 * p3 : 2 * p3 + 2],
                        rhs=h2s.pop(p3),
                        start=True,
                        stop=True,
                    )
                    gi = p3 % G
                    if gi == 0:
                        tmp = tmppool.tile([2, G * BL], FP, tag="tmp")
                    nc.vector.tensor_copy(tmp[:, gi * BL : (gi + 1) * BL], ps3)
                    if gi == G - 1:
                        g0 = p3 - G + 1  # first pair of the group
                        r0 = g0 % 128
                        cs = (g0 // 128) * BL
                        src_m = tmp[0:1, :].rearrange("a (g b) -> a g b", b=BL)
                        src_l = tmp[1:2, :].rearrange("a (g b) -> a g b", b=BL)
                        nc.sync.dma_start(stg_m[r0 : r0 + G, cs : cs + BL], src_m)
                        nc.sync.dma_start(stg_l[r0 : r0 + G, cs : cs + BL], src_l)

            # Tail: per column-block, bias-add mean and clamp logvar.
            b3m_t = consts.tile([128, NBLK], FP)
            sb1_t = consts.tile([128, NBLK], FP)
            for blk in range(NBLK):
                n = min(128, P - blk * 128)
                nc.sync.dma_start(
                    b3m_t[:n, blk : blk + 1], b3m_d[blk * 128 : blk * 128 + n]
                )
                nc.sync.dma_start(
                    sb1_t[:n, blk : blk + 1], sb1_d[blk * 128 : blk * 128 + n]
                )

            def emit_softplus(z, n, tag):
                # softplus(z) = max(z,0) + ln(1 + exp(-|z|))
                a = tailpool.tile([128, BL], FP, tag=f"{tag}_a")
                nc.vector.scalar_tensor_tensor(
                    a[:n], z, -1.0, z, ALU.mult, ALU.max
                )  # |z|
                e = tailpool.tile([128, BL], FP, tag=f"{tag}_e")
                nc.scalar.activation(e[:n], a[:n], AF.Exp, scale=-1.0)
                l = tailpool.tile([128, BL], FP, tag=f"{tag}_l")
                nc.scalar.activation(l[:n], e[:n], AF.Ln, bias=1.0, scale=1.0)
                mx = tailpool.tile([128, BL], FP, tag=f"{tag}_m")
                nc.vector.tensor_scalar_max(mx[:n], z, 0.0)
                o = tailpool.tile([128, BL], FP, tag=f"{tag}_o")
                nc.vector.tensor_add(o[:n], l[:n], mx[:n])
                return o[:n]

            for blk in range(NBLK):
                n = min(128, P - blk * 128)
                cs = blk * BL
                mout = tailpool.tile([128, BL], FP, tag="mout")
                nc.vector.tensor_scalar_add(
                    mout[:n], stg_m[:n, cs : cs + BL], b3m_t[:n, blk : blk + 1]
                )
                nc.sync.dma_start(mean_o[:n, cs : cs + BL], mout[:n])

                # z1 = (MAX - b3_lv) - lv
                z1 = tailpool.tile([128, BL], FP, tag="z1")
                nc.vector.tensor_scalar(
                    z1[:n],
                    stg_l[:n, cs : cs + BL],
                    -1.0,
                    sb1_t[:n, blk : blk + 1],
                    ALU.mult,
                    ALU.add,
                )
                t1 = emit_softplus(z1[:n], n, "sp1")
                # z2 = (MAX - t1) - MIN
                z2 = tailpool.tile([128, BL], FP, tag="z2")
                nc.vector.tensor_scalar(
                    z2[:n], t1, -1.0, MAX_LOGVAR - MIN_LOGVAR, ALU.mult, ALU.add
                )
                t3 = emit_softplus(z2[:n], n, "sp2")
                lvout = tailpool.tile([128, BL], FP, tag="lvout")
                nc.vector.tensor_scalar_add(lvout[:n], t3, MIN_LOGVAR)
                nc.sync.dma_start(lv_o[:n, cs : cs + BL], lvout[:n])

    nc.compile()
    return nc


def _get_nc():
    if "nc" not in _NC_CACHE:
        _NC_CACHE["nc"] = build_bass()
    return _NC_CACHE["nc"]


def host_prep(x, masks, W1, b1, W2, b2, W3, b3):
    """Numpy-side input massaging shared by kernel() and the simulator test."""
    f32 = np.float32
    x = np.asarray(x, f32)
    masks = np.asarray(masks, f32)
    W1 = np.asarray(W1, f32)
    b1 = np.asarray(b1, f32)
    W2 = np.asarray(W2, f32)
    b2 = np.asarray(b2, f32)
    W3 = np.asarray(W3, f32)
    b3 = np.asarray(b3, f32)

    m = masks.transpose(1, 0, 2)  # (D,E,IN)
    W1m = m[:, :, :, None] * W1  # (D,E,IN,H): (x*m)@W1 == x@(m*W1)
    W1a = np.concatenate([W1m, b1[:, :, None, :]], axis=2)  # (D,E,IN+1,H)
    w1 = np.ascontiguousarray(
        W1a.reshape(P, IN + 1, H).transpose(1, 0, 2).reshape(IN + 1, P * H)
    )
    w2 = np.ascontiguousarray(
        W2.reshape(P, H, H).transpose(1, 0, 2).reshape(H, P * H)
    )
    w3 = np.ascontiguousarray(
        W3.reshape(P, H, 2).transpose(1, 0, 2).reshape(H, 2 * P)
    )
    b2T = np.ascontiguousarray(b2.reshape(P, H).T)  # (H,P)
    b3r = b3.reshape(P, 2)
    b3m = np.ascontiguousarray(b3r[:, 0:1])  # (P,1)
    sb1 = np.ascontiguousarray(MAX_LOGVAR - b3r[:, 1:2])  # (P,1)

    xT = np.ascontiguousarray(x.T)  # (IN,B)
    per_core = []
    for c in range(NCORES):
        sl = xT[:, c * BL : (c + 1) * BL]
        xTa = np.concatenate([sl, np.ones((1, BL), f32)], axis=0)  # (IN+1,BL)
        per_core.append(np.ascontiguousarray(xTa))

    common = {"w1": w1, "w2": w2, "w3": w3, "b2T": b2T, "b3m": b3m, "sb1": sb1}
    return common, per_core


def assemble(core_means, core_lvs):
    """(128, NBLK*BL) staging dumps per core -> (mean, logvar), (D,E,nb,1)."""

    def unstage(arr):
        # pair p lives at [p % 128, (p // 128)*BL : ...]
        blocks = [arr[:, b * BL : (b + 1) * BL] for b in range(NBLK)]
        return np.concatenate(blocks, axis=0)[:P]  # (P, BL)

    mean = np.concatenate([unstage(a) for a in core_means], axis=1)  # (P, nb)
    lv = np.concatenate([unstage(a) for a in core_lvs], axis=1)
    nb = mean.shape[1]
    mean = mean.reshape(D, E, nb, 1).astype(np.float32)
    lv = lv.reshape(D, E, nb, 1).astype(np.float32)
    return mean, lv


def kernel(x, masks, W1, b1, W2, b2, W3, b3):
    global LAST_RESULT
    from concourse.bass_utils import run_bass_kernel_spmd

    common, per_core = host_prep(x, masks, W1, b1, W2, b2, W3, b3)
    nc = _get_nc()

    in_maps = [dict(common, xTa=per_core[c]) for c in range(NCORES)]
    res = run_bass_kernel_spmd(
        nc,
        in_maps,
        core_ids=list(range(NCORES)),
        trace=PROFILE,
    )
    LAST_RESULT = res

    return assemble(
        [r["mean"] for r in res.results], [r["lv"] for r in res.results]
    )
